# revision 2
# baseline (speedup 1.0000x reference)
"""MatchingNetwork forward on 8 Trainium2 NeuronCores.

The reference network's output reduces exactly to one_hot(labels, V) in f32:
the final einsum('btn,btv->btv', att, one_hot) sums att over n, and att is a
softmax over n, so the output is one_hot scaled by sum(softmax) == 1 (to float
rounding, ~1e-7).  Everything upstream (embedding gathers, BiLSTM GLayer,
attentional FLayer) cancels out of the result for every input.

So the kernel is a distributed one-hot materialization: B*T = 2048 rows of
V = 32000 f32 each (262 MB of output), data-parallel over rows across 8 cores
(256 rows/core = 32.77 MB/core of pure HBM writes -> memory-bound).

Per core: labels arrive as [128 partitions, 2] (row r = b*128 + p).  DVE
tensor_scalar(subtract, is_equal) compares iota row-chunks against the
per-partition label, producing 1.0/0.0 f32 tiles that stream to DRAM on the
two HWDGE queues (sync + scalar), which together saturate the 16-SDMA fabric
(~433 GB/s).  Startup is minimized with a ramp: small prefix-iota tiles
(512/1024 wide, generated first on gpsimd) let the first writes issue ~0.5 us
after the labels land, instead of waiting for a full-width iota; wide 2000-col
tiles use two compare sub-ops over prefix (0..1023) + suffix (1024..1999)
iota tiles so no wide iota generation gates the stream.  A small 464-wide
tail tile per batch lets both queues drain together.
"""

import os
import sys

for _p in ("/opt/trn_rl_repo", "/root/.axon_site/_ro/trn_rl_repo"):
    if os.path.isdir(_p) and _p not in sys.path:
        sys.path.append(_p)

import numpy as np

B, T, V = 32, 64, 32000
N_CORES = 8
ROWS = B * T                 # 2048 one-hot rows total
RPC = ROWS // N_CORES        # 256 rows per core
NB = RPC // 128              # 2 batches of 128 partitions

WIDE_W = 2000                # steady-state tile width (1 MB DMAs)
N_WIDE = 15
RAMP = [(0, 512), (512, 1024)]          # (col, width) startup ramp tiles
WIDE0 = 1536                            # first wide tile column
TAIL = (31536, 464)                     # small tail tile balances queue ends

_cache = {}


def _build_nc():
    import concourse.bacc as bacc
    import concourse.mybir as mybir
    from concourse.tile import TileContext

    f32 = mybir.dt.float32
    nc = bacc.Bacc()
    lab_d = nc.dram_tensor("labels", [128, NB], f32, kind="ExternalInput")
    out_d = nc.dram_tensor("out", [NB, 128, V], f32, kind="ExternalOutput")

    with TileContext(nc) as tc:
        with tc.tile_pool(name="const", bufs=1) as cpool, \
             tc.tile_pool(name="work", bufs=8) as wpool:
            lab = cpool.tile([128, NB], f32, tag="lab")
            nc.sync.dma_start(out=lab[:, :], in_=lab_d[:, :])
            # Prefix/suffix iota tiles; separate tiles make per-tile deps
            # exact so ramp compares only wait on what they read.
            rb = cpool.tile([128, 512], f32, tag="rb")    # values 0..511
            rc = cpool.tile([128, 1024], f32, tag="rc")   # values 0..1023
            rd = cpool.tile([128, 976], f32, tag="rd")    # values 1024..1999
            for t, w, base in ((rb, 512, 0), (rc, 1024, 0), (rd, 976, 1024)):
                nc.gpsimd.iota(t[:, :w], [[1, w]], base=base,
                               channel_multiplier=0,
                               allow_small_or_imprecise_dtypes=True)
            dma_engines = [nc.sync, nc.scalar]

            def emit(col, w, pieces):
                # pieces: (iota_ap, lo, hi) sub-ranges covering [0, w); the
                # iota holds values (global_col - col) over [lo, hi).
                for b in range(NB):
                    o = wpool.tile([128, WIDE_W], f32, tag="o")
                    for it, lo, hi in pieces:
                        # o = is_equal(iota - (-col), lab[:, b])
                        nc.vector.tensor_scalar(
                            out=o[:, lo:hi], in0=it,
                            scalar1=float(-col), scalar2=lab[:, b:b + 1],
                            op0=mybir.AluOpType.subtract,
                            op1=mybir.AluOpType.is_equal)
                    dma_engines[b].dma_start(
                        out=out_d[b, :, col:col + w], in_=o[:, :w])

            emit(0, 512, [(rb[:, :512], 0, 512)])
            emit(512, 1024, [(rc[:, :1024], 0, 1024)])
            for k in range(N_WIDE):
                c = WIDE0 + k * WIDE_W
                emit(c, WIDE_W, [(rc[:, :1024], 0, 1024),
                                 (rd[:, :976], 1024, 2000)])
            emit(TAIL[0], TAIL[1], [(rb[:, :TAIL[1]], 0, TAIL[1])])
    nc.finalize()
    return nc


def kernel(**inputs):
    from concourse.bass_utils import run_bass_kernel_spmd

    if "nc" not in _cache:
        _cache["nc"] = _build_nc()
    nc = _cache["nc"]

    # Label values < 2^24 are exact in f32.
    lab = np.asarray(inputs["labels"]).reshape(-1).astype(np.float32)
    in_maps = []
    for i in range(N_CORES):
        shard = lab[i * RPC:(i + 1) * RPC].reshape(NB, 128).T  # [128, NB]
        in_maps.append({"labels": np.ascontiguousarray(shard)})

    trace = bool(int(os.environ.get("BASS_KERNEL_TRACE", "0")))
    res = run_bass_kernel_spmd(nc, in_maps, list(range(N_CORES)), trace=trace)
    _cache["last_res"] = res

    outs = [res.results[i]["out"].reshape(RPC, V) for i in range(N_CORES)]
    return np.concatenate(outs, axis=0).reshape(B, T, V)
